# revision 4
# baseline (speedup 1.0000x reference)
"""HGT kernel for 8 trn2 NeuronCores — v2 (bf16, host sel-masks, overlap).

Changes vs v1:
  - bf16 node features / kv / q tables, weights, collectives (halves HBM+wire
    traffic); f32 PSUM accumulation and f32 softmax math on-chip.
  - x fed host-transposed (x_T [768, n]) so the input MLP loads lhsT tiles
    directly - no PE transposes in phase 0.
  - Selection matrices (segment-sum masks) precomputed on host and streamed
    as bf16 [128,128] blocks: drops the per-tile PE transpose + PSUM copy +
    is_equal chain of v1.
  - Phase C uses ONE matmul per tile: per-(dst,typ) rows [den|msg] scattered
    to outs_r2; softmax normalization deferred to phase E (dense).
  - Scratch DRAM (outs_r2, part_u) zeroed once up front; scatters overwrite
    the same rows every layer.
  - Phase A split movie+user first so both AllGathers overlap the review
    kqv matmuls; ReduceScatter overlaps review/movie phase E.
"""

import math
import numpy as np
import ml_dtypes

try:
    import concourse  # noqa
except ImportError:
    import sys
    sys.path.insert(0, "/opt/trn_rl_repo")

import sys as _sys, types as _types
try:
    import antenv.axon_hooks  # noqa
except Exception:
    try:
        import antenv
        _stub = _types.ModuleType("antenv.axon_hooks")
        _stub.get_axon_ntff_profile_hook = lambda: None
        _sys.modules["antenv.axon_hooks"] = _stub
        antenv.axon_hooks = _stub
    except Exception:
        pass

from concourse import bacc, bass, mybir, tile
from concourse.bass import IndirectOffsetOnAxis
from concourse.bass_utils import run_bass_kernel_spmd
from concourse.masks import make_identity

P = 128
H, DH, HID, IN_DIM, OUT_DIM = 8, 32, 256, 768, 128
L = 2
NU_F, NM_F, NR_F = 50000, 20000, 200000
C = 8
NU, NM, NR = NU_F // C, NM_F // C, NR_F // C  # 6250, 2500, 25000
AG_BLK = NM + NU  # 8750 rows per core in the kv allgather (movie first)
UBLK = 6400       # padded user block in the ReduceScatter partial
G = 8             # tiles per sel-mask/stream batch
F32 = mybir.dt.float32
BF16 = mybir.dt.bfloat16
I32 = mybir.dt.int32
AF = mybir.ActivationFunctionType
ALU = mybir.AluOpType
BF = ml_dtypes.bfloat16

LAST_RESULTS = None
LAST_NC = None


# ---------------------------------------------------------------- host prep

def _fold_weights(inp):
    Wk, bk = inp["Wk"], inp["bk"]
    Wq, bq = inp["Wq"], inp["bq"]
    Wv, bv = inp["Wv"], inp["bv"]
    Wa, ba = inp["Wa"], inp["ba"]
    a_rel, m_rel, p_rel, skip = inp["a_rel"], inp["m_rel"], inp["p_rel"], inp["skip"]
    s_of_e = {0: 1, 1: 0, 2: 2}
    out = {}
    for l in range(L):
        for e in range(3):
            s = s_of_e[e]
            wk_eff = np.empty((HID, HID), np.float32)
            bk_eff = np.empty((HID,), np.float32)
            wv_eff = np.empty((HID, HID), np.float32)
            bv_eff = np.empty((HID,), np.float32)
            for h in range(H):
                sl = slice(h * DH, (h + 1) * DH)
                sc = float(p_rel[l, e, h]) / math.sqrt(DH)
                wk_eff[:, sl] = (Wk[l, s][:, sl] @ a_rel[l, e, h]) * sc
                bk_eff[sl] = (bk[l, s][sl] @ a_rel[l, e, h]) * sc
                wv_eff[:, sl] = Wv[l, s][:, sl] @ m_rel[l, e, h]
                bv_eff[sl] = bv[l, s][sl] @ m_rel[l, e, h]
            out[f"wkv_t{s}_l{l}"] = np.concatenate([wk_eff, wv_eff], 1).astype(BF)
            out[f"bkv_t{s}_l{l}"] = np.concatenate(
                [bk_eff, bv_eff]).reshape(1, 512).astype(np.float32)
        for t in (0, 2):
            out[f"wq_t{t}_l{l}"] = np.ascontiguousarray(Wq[l, t]).astype(BF)
            out[f"bq_t{t}_l{l}"] = np.ascontiguousarray(
                bq[l, t]).reshape(1, HID).astype(np.float32)
        for t in range(3):
            g = 1.0 / (1.0 + math.exp(-float(skip[l, t])))
            out[f"omg_l{l}_t{t}"] = 1.0 - g
            if t != 1:
                out[f"wa_t{t}_l{l}"] = (np.ascontiguousarray(Wa[l, t]) * g).astype(BF)
            out[f"ba_t{t}_l{l}"] = (np.ascontiguousarray(ba[l, t]) * g
                                    ).reshape(1, HID).astype(np.float32)
    out["w1"] = np.ascontiguousarray(inp["W1"]).astype(BF)
    out["b1"] = inp["b1"].reshape(1, HID).astype(np.float32)
    out["w2"] = np.ascontiguousarray(inp["W2"]).astype(BF)
    out["b2"] = inp["b2"].reshape(1, OUT_DIM).astype(np.float32)
    return out


def _pack(group_ids, payload_cols, pad_vals, dtypes):
    """Pack edges (sorted by group) into 128-slot tiles; groups never straddle
    a tile. Also returns per-slot group ids (pad slots get unique ids) for
    sel-mask construction."""
    n = len(group_ids)
    order = np.argsort(group_ids, kind="stable")
    g = group_ids[order]
    uniq, counts = np.unique(g, return_counts=True)
    ng = len(uniq)
    tile_id = np.empty(ng, np.int64)
    slot0 = np.empty(ng, np.int64)
    cur_t, fill = 0, 0
    cl = counts.tolist()
    for i in range(ng):
        c = cl[i]
        assert c <= P
        if fill + c > P:
            cur_t += 1
            fill = 0
        tile_id[i] = cur_t
        slot0[i] = fill
        fill += c
    T = cur_t + 1
    gi = np.repeat(np.arange(ng), counts)
    starts = np.cumsum(counts) - counts
    within = np.arange(n) - starts[gi]
    tid = tile_id[gi]
    slot = slot0[gi] + within
    outs = []
    for col, pv, dt in zip(payload_cols, pad_vals, dtypes):
        arr = np.full((T, P), pv, dtype=dt)
        arr[tid, slot] = col[order].astype(dt)
        outs.append(arr)
    gids = np.full((T, P), -1, np.int64)
    gids[tid, slot] = gi
    pad_mask = gids < 0
    # unique negative ids for pad slots so they never match each other
    gids[pad_mask] = -(np.arange(np.count_nonzero(pad_mask)) + 2).astype(np.int64)
    return T, outs, gids


def _equalize(per_core, pad_vals, dtypes, batch=G):
    """Pad every core's [T,P] arrays to common T (multiple of batch).
    Returns T and per-core ([P,T] payloads, [T,P] gids)."""
    T = max(max(t for t, _, _ in per_core), 1)
    T = ((T + batch - 1) // batch) * batch
    res = []
    for _, arrs, gids in per_core:
        padded = []
        for a, pv, dt in zip(arrs, pad_vals, dtypes):
            full = np.full((T, P), pv, dtype=dt)
            full[: a.shape[0]] = a
            padded.append(np.ascontiguousarray(full.T))
        gfull = np.full((T, P), -1, np.int64)
        gfull[: gids.shape[0]] = gids
        m = gfull < 0
        gfull[m] = -(np.arange(np.count_nonzero(m)) + 2)
        res.append((padded, gfull))
    return T, res


def _sel_stream(gids):
    """[T,P] group ids -> batch-major bf16 sel stream [T//G, P, G*P]."""
    T = gids.shape[0]
    sel = (gids[:, :, None] == gids[:, None, :])
    sel = sel.reshape(T // G, G, P, P).transpose(0, 2, 1, 3)
    return np.ascontiguousarray(sel.reshape(T // G, P, G * P).astype(BF))


def _prep_edges(inp):
    src_mr, dst_mr = inp["src_mr"], inp["dst_mr"]
    src_ur, dst_ur = inp["src_ur"], inp["dst_ur"]
    src_ru, dst_ru = inp["src_ru"], inp["dst_ru"]

    # phase C: review-dst edges (mr typ0 + ur typ1), sharded by dst shard;
    # group key = 2*dst_local + typ -> scatter row in outs_r2
    sm = (src_mr // NM) * AG_BLK + (src_mr % NM)
    su = (src_ur // NU) * AG_BLK + NM + (src_ur % NU)
    src_all = np.concatenate([sm, su]).astype(np.int64)
    dst_all = np.concatenate([dst_mr, dst_ur]).astype(np.int64)
    typ_all = np.concatenate(
        [np.zeros(len(sm), np.int64), np.ones(len(su), np.int64)])
    csp = []
    pvC = [0, 0, 2 * NR]  # src, qi, scatter-row (pad -> dump row 2*NR)
    dtC = [np.int32, np.int32, np.int32]
    for c in range(C):
        m = (dst_all // NR) == c
        dl = dst_all[m] % NR
        key = dl * 2 + typ_all[m]
        cols = [src_all[m], dl, key]
        csp.append(_pack(key, cols, pvC, dtC))
    T_C, cs = _equalize(csp, pvC, dtC)

    # phase D: ru edges (review->user), sharded by src shard
    s64, d64 = src_ru.astype(np.int64), dst_ru.astype(np.int64)
    flat = (d64 // NU) * UBLK + (d64 % NU)
    rup = []
    pvD = [0, 0, NU]  # pad scatter -> row NU (inside core-0 padding gap)
    dtD = [np.int32, np.int32, np.int32]
    for c in range(C):
        m = (s64 // NR) == c
        cols = [s64[m] % NR, d64[m], flat[m]]
        rup.append(_pack(flat[m], cols, pvD, dtD))
    T_D, ru = _equalize(rup, pvD, dtD)
    return T_C, cs, T_D, ru


# ---------------------------------------------------------------- device

def _rows_of(n):
    return [(r0, min(P, n - r0)) for r0 in range(0, n, P)]


def build_program(T_C, T_D, omg):
    nc = bacc.Bacc("TRN2", target_bir_lowering=False, debug=False,
                   enable_asserts=False, num_devices=C)
    RG = [list(range(C))]

    def din(name, shape, dt=F32):
        return nc.dram_tensor(name, list(shape), dt, kind="ExternalInput")

    def dint(name, shape, dt=BF16, shared=False):
        return nc.dram_tensor(name, list(shape), dt, kind="Internal",
                              addr_space="Shared" if shared else "Local")

    x_u = din("x_u", (IN_DIM, NU), BF16)   # host-transposed
    x_m = din("x_m", (IN_DIM, NM), BF16)
    x_r = din("x_r", (IN_DIM, NR), BF16)
    cs_names = ["cs_src", "cs_qi", "cs_dst"]
    cs_d = [din(n, (P, T_C), I32) for n in cs_names]
    ru_names = ["ru_src", "ru_qi", "ru_dst"]
    ru_d = [din(n, (P, T_D), I32) for n in ru_names]
    selC_d = din("selC", (T_C // G, P, G * P), BF16)
    selD_d = din("selD", (T_D // G, P, G * P), BF16)
    w1 = din("w1", (IN_DIM, HID), BF16)
    b1 = din("b1", (1, HID))
    w2 = din("w2", (HID, OUT_DIM), BF16)
    b2 = din("b2", (1, OUT_DIM))
    wd, bd = {}, {}
    for l in range(L):
        for s in range(3):
            wd[f"wkv_t{s}_l{l}"] = din(f"wkv_t{s}_l{l}", (HID, 512), BF16)
            bd[f"bkv_t{s}_l{l}"] = din(f"bkv_t{s}_l{l}", (1, 512))
        for t in (0, 2):
            wd[f"wq_t{t}_l{l}"] = din(f"wq_t{t}_l{l}", (HID, HID), BF16)
            bd[f"bq_t{t}_l{l}"] = din(f"bq_t{t}_l{l}", (1, HID))
            wd[f"wa_t{t}_l{l}"] = din(f"wa_t{t}_l{l}", (HID, HID), BF16)
            bd[f"ba_t{t}_l{l}"] = din(f"ba_t{t}_l{l}", (1, HID))
        bd[f"ba_t1_l{l}"] = din(f"ba_t1_l{l}", (1, HID))
    y_u = nc.dram_tensor("y_u", [NU, OUT_DIM], F32, kind="ExternalOutput")
    y_m = nc.dram_tensor("y_m", [NM, OUT_DIM], F32, kind="ExternalOutput")
    y_r = nc.dram_tensor("y_r", [NR, OUT_DIM], F32, kind="ExternalOutput")

    xs = {t: [dint(f"xs_t{t}_s{s}", (n, HID)) for s in range(L + 1)]
          for t, n in ((0, NU), (1, NM), (2, NR))}
    kv_own = [dint(f"kv_own_l{l}", (AG_BLK, 512)) for l in range(L)]
    qu_own = [dint(f"qu_own_l{l}", (NU, HID)) for l in range(L)]
    kv_src = [dint(f"kv_src_l{l}", (C * AG_BLK, 512), shared=True)
              for l in range(L)]
    q_uf = [dint(f"q_uf_l{l}", (NU_F, HID), shared=True) for l in range(L)]
    kv_ru = [dint(f"kv_ru_l{l}", (NR, 512)) for l in range(L)]
    q_r = [dint(f"q_r_l{l}", (NR, HID)) for l in range(L)]
    outs_r2 = dint("outs_r2", (2 * NR + P, 264))       # shared across layers
    part_u = [dint(f"part_u_l{l}", (C * UBLK, 264)) for l in range(L)]
    red_u = [dint(f"red_u_l{l}", (UBLK, 264)) for l in range(L)]

    with tile.TileContext(nc) as tc:
        from contextlib import ExitStack
        _stk = ExitStack()
        wp = _stk.enter_context(tc.tile_pool(name="wp", bufs=1))

        def mk(shape, dt, name):
            return wp.tile(shape, dt, tag=name, name=name)

        ident = mk([P, P], BF16, "ident")
        make_identity(nc, ident[:, :])

        cs_sb = []
        for n, dr in zip(cs_names, cs_d):
            t_ = mk([P, T_C], I32, n + "_sb")
            nc.sync.dma_start(t_[:], dr.ap()[:, :])
            cs_sb.append(t_)
        ru_sb = []
        for n, dr in zip(ru_names, ru_d):
            t_ = mk([P, T_D], I32, n + "_sb")
            nc.sync.dma_start(t_[:], dr.ap()[:, :])
            ru_sb.append(t_)

        def load_w(dr, in_dim, out_w, name):
            ts = []
            for cch in range(in_dim // P):
                t_ = mk([P, out_w], BF16, f"{name}_c{cch}")
                nc.sync.dma_start(t_[:], dr.ap()[cch * P:(cch + 1) * P, :])
                ts.append(t_)
            return ts

        def load_b(dr, w, name):
            t_ = mk([P, w], F32, name)
            nc.sync.dma_start(t_[:], dr.ap()[0:1, :].to_broadcast([P, w]))
            return t_

        w1_s = load_w(w1, IN_DIM, HID, "w1s")
        b1_s = load_b(b1, HID, "b1s")
        w2_s = load_w(w2, HID, OUT_DIM, "w2s")
        b2_s = load_b(b2, OUT_DIM, "b2s")
        ws, bs = {}, {}
        for k, dr in wd.items():
            ws[k] = load_w(dr, HID, 512 if k.startswith("wkv") else HID, k + "s")
        for k, dr in bd.items():
            bs[k] = load_b(dr, 512 if k.startswith("bkv") else HID, k + "s")

        sb = _stk.enter_context(tc.tile_pool(name="sb", bufs=2))
        pp = _stk.enter_context(tc.tile_pool(name="pp", bufs=2, space="PSUM"))

        zt = mk([P, 16, 264], BF16, "zt")
        nc.vector.memset(zt[:], 0.0)

        def memset_dram(dr, nrows, w, tag):
            nfull = (nrows // P) * P
            v = dr.ap()[0:nfull, :].rearrange("(p a) f -> p a f", p=P)
            a_tot = nfull // P
            a0 = 0
            while a0 < a_tot:
                aa = min(16, a_tot - a0)
                nc.sync.dma_start(v[:, a0:a0 + aa, :], zt[:, 0:aa, 0:w])
                a0 += aa
            if nrows > nfull:
                r = nrows - nfull
                nc.sync.dma_start(dr.ap()[nfull:nrows, :], zt[0:r, 0, 0:w])

        # zero scratch once; scatters overwrite the same rows each layer
        memset_dram(outs_r2, 2 * NR + P, 264, "z1")
        for l in range(L):
            memset_dram(part_u[l], C * UBLK, 264, f"z2{l}")

        def transposed(xt_ap, sz, nch, tag):
            outs = []
            for cch in range(nch):
                tp = pp.tile([P, P], BF16, tag="tp")
                nc.tensor.transpose(
                    out=tp[:, 0:sz],
                    in_=xt_ap[0:sz, cch * P:(cch + 1) * P],
                    identity=ident[0:sz, 0:sz])
                ts = sb.tile([P, P], BF16, tag=f"dts{cch}")
                nc.vector.tensor_copy(ts[:, 0:sz], tp[:, 0:sz])
                outs.append(ts)
            return outs

        def dense(x_dr, nrows, in_dim, jobs, tag):
            """x stored row-major [nrows, in_dim] bf16; PE-transpose tiles."""
            nch = in_dim // P
            for r0, sz in _rows_of(nrows):
                xt = sb.tile([P, in_dim], BF16, tag="dx")
                nc.sync.dma_start(xt[0:sz], x_dr.ap()[r0:r0 + sz, :])
                xT = transposed(xt, sz, nch, tag)
                for wt, finish in jobs:
                    ow = wt[0].shape[-1]
                    ps = pp.tile([P, ow], F32, tag="ps")
                    for cch in range(nch):
                        nc.tensor.matmul(out=ps[0:sz], lhsT=xT[cch][:, 0:sz],
                                         rhs=wt[cch][:],
                                         start=(cch == 0), stop=(cch == nch - 1))
                    finish(ps, r0, sz)

        def dense_T(xT_dr, nrows, in_dim, jobs, tag):
            """x stored TRANSPOSED [in_dim, nrows] bf16; lhsT loads direct."""
            nch = in_dim // P
            for r0, sz in _rows_of(nrows):
                xT = []
                for cch in range(nch):
                    t_ = sb.tile([P, P], BF16, tag=f"dxt{cch}")
                    nc.sync.dma_start(
                        t_[:, 0:sz],
                        xT_dr.ap()[cch * P:(cch + 1) * P, r0:r0 + sz])
                    xT.append(t_)
                for wt, finish in jobs:
                    ow = wt[0].shape[-1]
                    ps = pp.tile([P, ow], F32, tag="ps")
                    for cch in range(nch):
                        nc.tensor.matmul(out=ps[0:sz], lhsT=xT[cch][:, 0:sz],
                                         rhs=wt[cch][:],
                                         start=(cch == 0), stop=(cch == nch - 1))
                    finish(ps, r0, sz)

        def fin_store(bias_t, act, out_dr, off, ow, tag, alpha=0.0,
                      out_f32=False):
            def f(ps, r0, sz):
                ot = sb.tile([P, ow], F32 if out_f32 else BF16, tag="do")
                nc.vector.tensor_add(ot[0:sz], ps[0:sz], bias_t[0:sz, :])
                if act is not None:
                    nc.scalar.activation(out=ot[0:sz], in_=ot[0:sz], func=act,
                                         alpha=alpha)
                nc.sync.dma_start(out_dr.ap()[off + r0: off + r0 + sz, :],
                                  ot[0:sz])
            return f

        # ---- phase 0: input MLP (host-transposed x, no PE transposes)
        for t, x_dr, n in ((0, x_u, NU), (1, x_m, NM), (2, x_r, NR)):
            dense_T(x_dr, n, IN_DIM,
                    [(w1_s, fin_store(b1_s, AF.Lrelu, xs[t][0], 0, HID,
                                      f"p0t{t}", alpha=0.01))], f"p0t{t}")

        def edge_phase(T_n, idx_sb, sel_dr, kv_tab, q_tab, out_tab, tag):
            src_sb, qi_sb, dst_sb = idx_sb
            for b0 in range(T_n // G):
                selt = sb.tile([P, G, P], BF16, tag="selt")
                nc.sync.dma_start(
                    selt[:].rearrange("p g q -> p (g q)"),
                    sel_dr.ap()[b0, :, :])
                for gi_ in range(G):
                    tj = b0 * G + gi_
                    kvg = sb.tile([P, 512], BF16, tag="kv")
                    nc.gpsimd.indirect_dma_start(
                        out=kvg[:], out_offset=None, in_=kv_tab.ap(),
                        in_offset=IndirectOffsetOnAxis(
                            ap=src_sb[:, tj:tj + 1], axis=0))
                    qg = sb.tile([P, HID], BF16, tag="q")
                    nc.gpsimd.indirect_dma_start(
                        out=qg[:], out_offset=None, in_=q_tab.ap(),
                        in_offset=IndirectOffsetOnAxis(
                            ap=qi_sb[:, tj:tj + 1], axis=0))
                    kq = sb.tile([P, HID], F32, tag="kq")
                    nc.vector.tensor_mul(kq[:], kvg[:, 0:HID], qg[:])
                    lg = sb.tile([P, H], F32, tag="lg")
                    nc.vector.tensor_reduce(
                        out=lg[:], in_=kq[:].rearrange("p (h d) -> p h d", h=H),
                        axis=mybir.AxisListType.X, op=ALU.add)
                    rhs = sb.tile([P, 264], BF16, tag="rhs")
                    nc.scalar.activation(out=rhs[:, 0:H], in_=lg[:], func=AF.Exp)
                    nc.vector.tensor_tensor(
                        out=rhs[:, H:264].rearrange("p (h d) -> p h d", h=H),
                        in0=kvg[:, HID:512].rearrange("p (h d) -> p h d", h=H),
                        in1=rhs[:, 0:H].rearrange("p (h o) -> p h o", h=H)
                            .to_broadcast([P, H, DH]),
                        op=ALU.mult)
                    ssum = pp.tile([P, 512], F32, tag="es")
                    nc.tensor.matmul(out=ssum[:, 0:264], lhsT=selt[:, gi_, :],
                                     rhs=rhs[:], start=True, stop=True)
                    mo = sb.tile([P, 264], BF16, tag="mo")
                    nc.scalar.activation(out=mo[:], in_=ssum[:, 0:264],
                                         func=AF.Copy)
                    nc.gpsimd.indirect_dma_start(
                        out=out_tab.ap(), in_=mo[:],
                        out_offset=IndirectOffsetOnAxis(
                            ap=dst_sb[:, tj:tj + 1], axis=0),
                        in_offset=None)

        for l in range(L):
            # ---- phase A part 1: movie + user kqv (small), then AGs
            dense(xs[0][l], NU, HID, [
                (ws[f"wkv_t0_l{l}"], fin_store(bs[f"bkv_t0_l{l}"], None,
                                               kv_own[l], NM, 512, f"au{l}")),
                (ws[f"wq_t0_l{l}"], fin_store(bs[f"bq_t0_l{l}"], None,
                                              qu_own[l], 0, HID, f"aq{l}")),
            ], f"au{l}")
            dense(xs[1][l], NM, HID, [
                (ws[f"wkv_t1_l{l}"], fin_store(bs[f"bkv_t1_l{l}"], None,
                                               kv_own[l], 0, 512, f"am{l}")),
            ], f"am{l}")
            nc.gpsimd.collective_compute(
                "AllGather", ALU.bypass, replica_groups=RG,
                ins=[kv_own[l].ap()], outs=[kv_src[l].ap()])
            nc.gpsimd.collective_compute(
                "AllGather", ALU.bypass, replica_groups=RG,
                ins=[qu_own[l].ap()], outs=[q_uf[l].ap()])
            # ---- phase A part 2: review kqv (overlaps the AllGathers)
            dense(xs[2][l], NR, HID, [
                (ws[f"wkv_t2_l{l}"], fin_store(bs[f"bkv_t2_l{l}"], None,
                                               kv_ru[l], 0, 512, f"ar{l}")),
                (ws[f"wq_t2_l{l}"], fin_store(bs[f"bq_t2_l{l}"], None,
                                              q_r[l], 0, HID, f"arq{l}")),
            ], f"ar{l}")

            # ---- phase C: review-dst edge tiles (one matmul per tile)
            edge_phase(T_C, cs_sb, selC_d, kv_src[l], q_r[l], outs_r2, f"c{l}")
            # ---- phase D: ru edge tiles
            edge_phase(T_D, ru_sb, selD_d, kv_ru[l], q_uf[l], part_u[l],
                       f"d{l}")

            nc.gpsimd.collective_compute(
                "ReduceScatter", ALU.add, replica_groups=RG,
                ins=[part_u[l].ap()], outs=[red_u[l].ap()])

            # ---- phase E (review first: overlaps the ReduceScatter)
            def fin_blend(bias_t, xs_in, xs_out, t, tag):
                og = omg[(l, t)]
                def f(ps, r0, sz):
                    ot = sb.tile([P, HID], F32, tag="do")
                    nc.vector.tensor_add(ot[0:sz], ps[0:sz], bias_t[0:sz, :])
                    xt2 = sb.tile([P, HID], BF16, tag="dx2")
                    nc.sync.dma_start(xt2[0:sz], xs_in.ap()[r0:r0 + sz, :])
                    ob = sb.tile([P, HID], F32, tag="ob")
                    nc.vector.tensor_scalar_mul(
                        out=ob[0:sz], in0=xt2[0:sz], scalar1=og)
                    nc.vector.tensor_add(ot[0:sz], ot[0:sz], ob[0:sz])
                    oc = sb.tile([P, HID], BF16, tag="oc")
                    nc.vector.tensor_copy(oc[0:sz], ot[0:sz])
                    nc.sync.dma_start(xs_out.ap()[r0:r0 + sz, :], oc[0:sz])
                return f

            def rev_att(r0, sz, tag):
                rt = sb.tile([P, 528], BF16, tag="er")
                nc.sync.dma_start(
                    rt[0:sz],
                    outs_r2.ap()[2 * r0:2 * (r0 + sz), :]
                        .rearrange("(n two) f -> n (two f)", two=2))
                dn = sb.tile([P, 2 * H], F32, tag="edn")
                nc.vector.tensor_scalar_add(
                    out=dn[0:sz].rearrange("p (two f) -> p two f", two=2),
                    in0=rt[0:sz].rearrange("p (two f) -> p two f", two=2)
                        [:, :, 0:H],
                    scalar1=1e-16)
                rd = sb.tile([P, 2 * H], F32, tag="erd")
                nc.vector.reciprocal(out=rd[0:sz], in_=dn[0:sz])
                at = sb.tile([P, HID], F32, tag="ea")
                nc.vector.tensor_tensor(
                    out=at[0:sz].rearrange("p (h d) -> p h d", h=H),
                    in0=rt[0:sz, H:264].rearrange("p (h d) -> p h d", h=H),
                    in1=rd[0:sz, 0:H].rearrange("p (h o) -> p h o", h=H)
                        .to_broadcast([sz, H, DH]),
                    op=ALU.mult)
                a2 = sb.tile([P, HID], F32, tag="ea2")
                nc.vector.tensor_tensor(
                    out=a2[0:sz].rearrange("p (h d) -> p h d", h=H),
                    in0=rt[0:sz, 264 + H:528].rearrange("p (h d) -> p h d", h=H),
                    in1=rd[0:sz, H:2 * H].rearrange("p (h o) -> p h o", h=H)
                        .to_broadcast([sz, H, DH]),
                    op=ALU.mult)
                nc.vector.tensor_add(at[0:sz], at[0:sz], a2[0:sz])
                ag = sb.tile([P, HID], BF16, tag="eag")
                nc.scalar.activation(out=ag[0:sz], in_=at[0:sz], func=AF.Gelu)
                return ag

            def user_att(r0, sz, tag):
                rt = sb.tile([P, 264], BF16, tag="eru")
                nc.sync.dma_start(rt[0:sz], red_u[l].ap()[r0:r0 + sz, :])
                dn = sb.tile([P, H], F32, tag="ednu")
                nc.vector.tensor_scalar_add(
                    out=dn[0:sz], in0=rt[0:sz, 0:H], scalar1=1e-16)
                rd = sb.tile([P, H], F32, tag="erdu")
                nc.vector.reciprocal(out=rd[0:sz], in_=dn[0:sz])
                at = sb.tile([P, HID], F32, tag="eau")
                nc.vector.tensor_tensor(
                    out=at[0:sz].rearrange("p (h d) -> p h d", h=H),
                    in0=rt[0:sz, H:264].rearrange("p (h d) -> p h d", h=H),
                    in1=rd[0:sz].rearrange("p (h o) -> p h o", h=H)
                        .to_broadcast([sz, H, DH]),
                    op=ALU.mult)
                ag = sb.tile([P, HID], BF16, tag="eagu")
                nc.scalar.activation(out=ag[0:sz], in_=at[0:sz], func=AF.Gelu)
                return ag

            for t, n, attf in ((2, NR, rev_att), (0, NU, user_att)):
                wt = ws[f"wa_t{t}_l{l}"]
                fin = fin_blend(bs[f"ba_t{t}_l{l}"], xs[t][l], xs[t][l + 1],
                                t, f"e{t}{l}")
                for r0, sz in _rows_of(n):
                    at = attf(r0, sz, f"e{t}{l}")
                    xT = transposed(at, sz, HID // P, f"e{t}{l}")
                    ps = pp.tile([P, HID], F32, tag="ps2")
                    for cch in range(HID // P):
                        nc.tensor.matmul(out=ps[0:sz], lhsT=xT[cch][:, 0:sz],
                                         rhs=wt[cch][:],
                                         start=(cch == 0), stop=(cch == 1))
                    fin(ps, r0, sz)
            og = omg[(l, 1)]
            bam = bs[f"ba_t1_l{l}"]
            for r0, sz in _rows_of(NM):
                xt = sb.tile([P, HID], BF16, tag="em")
                nc.sync.dma_start(xt[0:sz], xs[1][l].ap()[r0:r0 + sz, :])
                ot = sb.tile([P, HID], F32, tag="em2")
                nc.vector.tensor_scalar_mul(
                    out=ot[0:sz], in0=xt[0:sz], scalar1=og)
                nc.vector.tensor_add(ot[0:sz], ot[0:sz], bam[0:sz, :])
                oc = sb.tile([P, HID], BF16, tag="em3")
                nc.vector.tensor_copy(oc[0:sz], ot[0:sz])
                nc.sync.dma_start(xs[1][l + 1].ap()[r0:r0 + sz, :], oc[0:sz])

        # ---- phase F: output MLP
        for t, y_dr, n in ((0, y_u, NU), (1, y_m, NM), (2, y_r, NR)):
            dense(xs[t][L], n, HID,
                  [(w2_s, fin_store(b2_s, AF.Lrelu, y_dr, 0, OUT_DIM,
                                    f"pft{t}", alpha=0.01, out_f32=True))],
                  f"pft{t}")
        _stk.close()

    nc.finalize()
    return nc


# ---------------------------------------------------------------- entry

_CACHE = {}


def kernel(**inputs):
    import os
    inp = {k: np.asarray(v) for k, v in inputs.items()}
    w = _fold_weights(inp)
    T_C, cs, T_D, ru = _prep_edges(inp)
    omg = {(l, t): w[f"omg_l{l}_t{t}"] for l in range(L) for t in range(3)}

    key = (T_C, T_D)
    if key not in _CACHE:
        _CACHE[key] = build_program(T_C, T_D, omg)
    nc = _CACHE[key]

    cs_names = ["cs_src", "cs_qi", "cs_dst"]
    ru_names = ["ru_src", "ru_qi", "ru_dst"]
    in_maps = []
    for c in range(C):
        m = {
            "x_u": np.ascontiguousarray(
                inp["x_user"][c * NU:(c + 1) * NU].T).astype(BF),
            "x_m": np.ascontiguousarray(
                inp["x_movie"][c * NM:(c + 1) * NM].T).astype(BF),
            "x_r": np.ascontiguousarray(
                inp["x_review"][c * NR:(c + 1) * NR].T).astype(BF),
            "w1": w["w1"], "b1": w["b1"], "w2": w["w2"], "b2": w["b2"],
        }
        arrs, gids = cs[c]
        for n, a in zip(cs_names, arrs):
            m[n] = a
        m["selC"] = _sel_stream(gids)
        arrs, gids = ru[c]
        for n, a in zip(ru_names, arrs):
            m[n] = a
        m["selD"] = _sel_stream(gids)
        for l in range(L):
            for s in range(3):
                m[f"wkv_t{s}_l{l}"] = w[f"wkv_t{s}_l{l}"]
                m[f"bkv_t{s}_l{l}"] = w[f"bkv_t{s}_l{l}"]
            for t in (0, 2):
                for nme in (f"wq_t{t}_l{l}", f"bq_t{t}_l{l}",
                            f"wa_t{t}_l{l}", f"ba_t{t}_l{l}"):
                    m[nme] = w[nme]
            m[f"ba_t1_l{l}"] = w[f"ba_t1_l{l}"]
        in_maps.append(m)

    trace = os.environ.get("BASS_KERNEL_TRACE") == "1"
    res = run_bass_kernel_spmd(nc, in_maps, core_ids=list(range(C)),
                               trace=trace)
    global LAST_RESULTS, LAST_NC
    LAST_RESULTS = res
    LAST_NC = nc
    r = res.results
    yu = np.concatenate([r[c]["y_u"] for c in range(C)], 0)
    ym = np.concatenate([r[c]["y_m"] for c in range(C)], 0)
    yr = np.concatenate([r[c]["y_r"] for c in range(C)], 0)
    return np.concatenate([yu, ym, yr], 0).astype(np.float32)


# revision 5
# speedup vs baseline: 1.2001x; 1.2001x over previous
"""HGT kernel for 8 trn2 NeuronCores — v2 (bf16, host sel-masks, overlap).

v3: per-batch (G=8) fused kq/reduce/exp DVE+ACT ops; D-phase before C;
AllGather q before kv. Changes vs v1:
  - bf16 node features / kv / q tables, weights, collectives (halves HBM+wire
    traffic); f32 PSUM accumulation and f32 softmax math on-chip.
  - x fed host-transposed (x_T [768, n]) so the input MLP loads lhsT tiles
    directly - no PE transposes in phase 0.
  - Selection matrices (segment-sum masks) precomputed on host and streamed
    as bf16 [128,128] blocks: drops the per-tile PE transpose + PSUM copy +
    is_equal chain of v1.
  - Phase C uses ONE matmul per tile: per-(dst,typ) rows [den|msg] scattered
    to outs_r2; softmax normalization deferred to phase E (dense).
  - Scratch DRAM (outs_r2, part_u) zeroed once up front; scatters overwrite
    the same rows every layer.
  - Phase A split movie+user first so both AllGathers overlap the review
    kqv matmuls; ReduceScatter overlaps review/movie phase E.
"""

import math
import numpy as np
import ml_dtypes

try:
    import concourse  # noqa
except ImportError:
    import sys
    sys.path.insert(0, "/opt/trn_rl_repo")

import sys as _sys, types as _types
try:
    import antenv.axon_hooks  # noqa
except Exception:
    try:
        import antenv
        _stub = _types.ModuleType("antenv.axon_hooks")
        _stub.get_axon_ntff_profile_hook = lambda: None
        _sys.modules["antenv.axon_hooks"] = _stub
        antenv.axon_hooks = _stub
    except Exception:
        pass

from concourse import bacc, bass, mybir, tile
from concourse.bass import IndirectOffsetOnAxis
from concourse.bass_utils import run_bass_kernel_spmd
from concourse.masks import make_identity

P = 128
H, DH, HID, IN_DIM, OUT_DIM = 8, 32, 256, 768, 128
L = 2
NU_F, NM_F, NR_F = 50000, 20000, 200000
C = 8
NU, NM, NR = NU_F // C, NM_F // C, NR_F // C  # 6250, 2500, 25000
AG_BLK = NM + NU  # 8750 rows per core in the kv allgather (movie first)
UBLK = 6400       # padded user block in the ReduceScatter partial
G = 8             # tiles per sel-mask/stream batch
F32 = mybir.dt.float32
BF16 = mybir.dt.bfloat16
I32 = mybir.dt.int32
AF = mybir.ActivationFunctionType
ALU = mybir.AluOpType
BF = ml_dtypes.bfloat16

LAST_RESULTS = None
LAST_NC = None


# ---------------------------------------------------------------- host prep

def _fold_weights(inp):
    Wk, bk = inp["Wk"], inp["bk"]
    Wq, bq = inp["Wq"], inp["bq"]
    Wv, bv = inp["Wv"], inp["bv"]
    Wa, ba = inp["Wa"], inp["ba"]
    a_rel, m_rel, p_rel, skip = inp["a_rel"], inp["m_rel"], inp["p_rel"], inp["skip"]
    s_of_e = {0: 1, 1: 0, 2: 2}
    out = {}
    for l in range(L):
        for e in range(3):
            s = s_of_e[e]
            wk_eff = np.empty((HID, HID), np.float32)
            bk_eff = np.empty((HID,), np.float32)
            wv_eff = np.empty((HID, HID), np.float32)
            bv_eff = np.empty((HID,), np.float32)
            for h in range(H):
                sl = slice(h * DH, (h + 1) * DH)
                sc = float(p_rel[l, e, h]) / math.sqrt(DH)
                wk_eff[:, sl] = (Wk[l, s][:, sl] @ a_rel[l, e, h]) * sc
                bk_eff[sl] = (bk[l, s][sl] @ a_rel[l, e, h]) * sc
                wv_eff[:, sl] = Wv[l, s][:, sl] @ m_rel[l, e, h]
                bv_eff[sl] = bv[l, s][sl] @ m_rel[l, e, h]
            out[f"wkv_t{s}_l{l}"] = np.concatenate([wk_eff, wv_eff], 1).astype(BF)
            out[f"bkv_t{s}_l{l}"] = np.concatenate(
                [bk_eff, bv_eff]).reshape(1, 512).astype(np.float32)
        for t in (0, 2):
            out[f"wq_t{t}_l{l}"] = np.ascontiguousarray(Wq[l, t]).astype(BF)
            out[f"bq_t{t}_l{l}"] = np.ascontiguousarray(
                bq[l, t]).reshape(1, HID).astype(np.float32)
        for t in range(3):
            g = 1.0 / (1.0 + math.exp(-float(skip[l, t])))
            out[f"omg_l{l}_t{t}"] = 1.0 - g
            if t != 1:
                out[f"wa_t{t}_l{l}"] = (np.ascontiguousarray(Wa[l, t]) * g).astype(BF)
            out[f"ba_t{t}_l{l}"] = (np.ascontiguousarray(ba[l, t]) * g
                                    ).reshape(1, HID).astype(np.float32)
    out["w1"] = np.ascontiguousarray(inp["W1"]).astype(BF)
    out["b1"] = inp["b1"].reshape(1, HID).astype(np.float32)
    out["w2"] = np.ascontiguousarray(inp["W2"]).astype(BF)
    out["b2"] = inp["b2"].reshape(1, OUT_DIM).astype(np.float32)
    return out


def _pack(group_ids, payload_cols, pad_vals, dtypes):
    """Pack edges (sorted by group) into 128-slot tiles; groups never straddle
    a tile. Also returns per-slot group ids (pad slots get unique ids) for
    sel-mask construction."""
    n = len(group_ids)
    order = np.argsort(group_ids, kind="stable")
    g = group_ids[order]
    uniq, counts = np.unique(g, return_counts=True)
    ng = len(uniq)
    tile_id = np.empty(ng, np.int64)
    slot0 = np.empty(ng, np.int64)
    cur_t, fill = 0, 0
    cl = counts.tolist()
    for i in range(ng):
        c = cl[i]
        assert c <= P
        if fill + c > P:
            cur_t += 1
            fill = 0
        tile_id[i] = cur_t
        slot0[i] = fill
        fill += c
    T = cur_t + 1
    gi = np.repeat(np.arange(ng), counts)
    starts = np.cumsum(counts) - counts
    within = np.arange(n) - starts[gi]
    tid = tile_id[gi]
    slot = slot0[gi] + within
    outs = []
    for col, pv, dt in zip(payload_cols, pad_vals, dtypes):
        arr = np.full((T, P), pv, dtype=dt)
        arr[tid, slot] = col[order].astype(dt)
        outs.append(arr)
    gids = np.full((T, P), -1, np.int64)
    gids[tid, slot] = gi
    pad_mask = gids < 0
    # unique negative ids for pad slots so they never match each other
    gids[pad_mask] = -(np.arange(np.count_nonzero(pad_mask)) + 2).astype(np.int64)
    return T, outs, gids


def _equalize(per_core, pad_vals, dtypes, batch=G):
    """Pad every core's [T,P] arrays to common T (multiple of batch).
    Returns T and per-core ([P,T] payloads, [T,P] gids)."""
    T = max(max(t for t, _, _ in per_core), 1)
    T = ((T + batch - 1) // batch) * batch
    res = []
    for _, arrs, gids in per_core:
        padded = []
        for a, pv, dt in zip(arrs, pad_vals, dtypes):
            full = np.full((T, P), pv, dtype=dt)
            full[: a.shape[0]] = a
            padded.append(np.ascontiguousarray(full.T))
        gfull = np.full((T, P), -1, np.int64)
        gfull[: gids.shape[0]] = gids
        m = gfull < 0
        gfull[m] = -(np.arange(np.count_nonzero(m)) + 2)
        res.append((padded, gfull))
    return T, res


def _sel_stream(gids):
    """[T,P] group ids -> batch-major bf16 sel stream [T//G, P, G*P]."""
    T = gids.shape[0]
    sel = (gids[:, :, None] == gids[:, None, :])
    sel = sel.reshape(T // G, G, P, P).transpose(0, 2, 1, 3)
    return np.ascontiguousarray(sel.reshape(T // G, P, G * P).astype(BF))


def _prep_edges(inp):
    src_mr, dst_mr = inp["src_mr"], inp["dst_mr"]
    src_ur, dst_ur = inp["src_ur"], inp["dst_ur"]
    src_ru, dst_ru = inp["src_ru"], inp["dst_ru"]

    # phase C: review-dst edges (mr typ0 + ur typ1), sharded by dst shard;
    # group key = 2*dst_local + typ -> scatter row in outs_r2
    sm = (src_mr // NM) * AG_BLK + (src_mr % NM)
    su = (src_ur // NU) * AG_BLK + NM + (src_ur % NU)
    src_all = np.concatenate([sm, su]).astype(np.int64)
    dst_all = np.concatenate([dst_mr, dst_ur]).astype(np.int64)
    typ_all = np.concatenate(
        [np.zeros(len(sm), np.int64), np.ones(len(su), np.int64)])
    csp = []
    pvC = [0, 0, 2 * NR]  # src, qi, scatter-row (pad -> dump row 2*NR)
    dtC = [np.int32, np.int32, np.int32]
    for c in range(C):
        m = (dst_all // NR) == c
        dl = dst_all[m] % NR
        key = dl * 2 + typ_all[m]
        cols = [src_all[m], dl, key]
        csp.append(_pack(key, cols, pvC, dtC))
    T_C, cs = _equalize(csp, pvC, dtC)

    # phase D: ru edges (review->user), sharded by src shard
    s64, d64 = src_ru.astype(np.int64), dst_ru.astype(np.int64)
    flat = (d64 // NU) * UBLK + (d64 % NU)
    rup = []
    pvD = [0, 0, NU]  # pad scatter -> row NU (inside core-0 padding gap)
    dtD = [np.int32, np.int32, np.int32]
    for c in range(C):
        m = (s64 // NR) == c
        cols = [s64[m] % NR, d64[m], flat[m]]
        rup.append(_pack(flat[m], cols, pvD, dtD))
    T_D, ru = _equalize(rup, pvD, dtD)
    return T_C, cs, T_D, ru


# ---------------------------------------------------------------- device

def _rows_of(n):
    return [(r0, min(P, n - r0)) for r0 in range(0, n, P)]


def build_program(T_C, T_D, omg):
    nc = bacc.Bacc("TRN2", target_bir_lowering=False, debug=False,
                   enable_asserts=False, num_devices=C)
    RG = [list(range(C))]

    def din(name, shape, dt=F32):
        return nc.dram_tensor(name, list(shape), dt, kind="ExternalInput")

    def dint(name, shape, dt=BF16, shared=False):
        return nc.dram_tensor(name, list(shape), dt, kind="Internal",
                              addr_space="Shared" if shared else "Local")

    x_u = din("x_u", (IN_DIM, NU), BF16)   # host-transposed
    x_m = din("x_m", (IN_DIM, NM), BF16)
    x_r = din("x_r", (IN_DIM, NR), BF16)
    cs_names = ["cs_src", "cs_qi", "cs_dst"]
    cs_d = [din(n, (P, T_C), I32) for n in cs_names]
    ru_names = ["ru_src", "ru_qi", "ru_dst"]
    ru_d = [din(n, (P, T_D), I32) for n in ru_names]
    selC_d = din("selC", (T_C // G, P, G * P), BF16)
    selD_d = din("selD", (T_D // G, P, G * P), BF16)
    w1 = din("w1", (IN_DIM, HID), BF16)
    b1 = din("b1", (1, HID))
    w2 = din("w2", (HID, OUT_DIM), BF16)
    b2 = din("b2", (1, OUT_DIM))
    wd, bd = {}, {}
    for l in range(L):
        for s in range(3):
            wd[f"wkv_t{s}_l{l}"] = din(f"wkv_t{s}_l{l}", (HID, 512), BF16)
            bd[f"bkv_t{s}_l{l}"] = din(f"bkv_t{s}_l{l}", (1, 512))
        for t in (0, 2):
            wd[f"wq_t{t}_l{l}"] = din(f"wq_t{t}_l{l}", (HID, HID), BF16)
            bd[f"bq_t{t}_l{l}"] = din(f"bq_t{t}_l{l}", (1, HID))
            wd[f"wa_t{t}_l{l}"] = din(f"wa_t{t}_l{l}", (HID, HID), BF16)
            bd[f"ba_t{t}_l{l}"] = din(f"ba_t{t}_l{l}", (1, HID))
        bd[f"ba_t1_l{l}"] = din(f"ba_t1_l{l}", (1, HID))
    y_u = nc.dram_tensor("y_u", [NU, OUT_DIM], F32, kind="ExternalOutput")
    y_m = nc.dram_tensor("y_m", [NM, OUT_DIM], F32, kind="ExternalOutput")
    y_r = nc.dram_tensor("y_r", [NR, OUT_DIM], F32, kind="ExternalOutput")

    xs = {t: [dint(f"xs_t{t}_s{s}", (n, HID)) for s in range(L + 1)]
          for t, n in ((0, NU), (1, NM), (2, NR))}
    kv_own = [dint(f"kv_own_l{l}", (AG_BLK, 512)) for l in range(L)]
    qu_own = [dint(f"qu_own_l{l}", (NU, HID)) for l in range(L)]
    kv_src = [dint(f"kv_src_l{l}", (C * AG_BLK, 512), shared=True)
              for l in range(L)]
    q_uf = [dint(f"q_uf_l{l}", (NU_F, HID), shared=True) for l in range(L)]
    kv_ru = [dint(f"kv_ru_l{l}", (NR, 512)) for l in range(L)]
    q_r = [dint(f"q_r_l{l}", (NR, HID)) for l in range(L)]
    outs_r2 = dint("outs_r2", (2 * NR + P, 264))       # shared across layers
    part_u = [dint(f"part_u_l{l}", (C * UBLK, 264)) for l in range(L)]
    red_u = [dint(f"red_u_l{l}", (UBLK, 264)) for l in range(L)]

    with tile.TileContext(nc) as tc:
        from contextlib import ExitStack
        _stk = ExitStack()
        wp = _stk.enter_context(tc.tile_pool(name="wp", bufs=1))

        def mk(shape, dt, name):
            return wp.tile(shape, dt, tag=name, name=name)

        ident = mk([P, P], BF16, "ident")
        make_identity(nc, ident[:, :])

        cs_sb = []
        for n, dr in zip(cs_names, cs_d):
            t_ = mk([P, T_C], I32, n + "_sb")
            nc.sync.dma_start(t_[:], dr.ap()[:, :])
            cs_sb.append(t_)
        ru_sb = []
        for n, dr in zip(ru_names, ru_d):
            t_ = mk([P, T_D], I32, n + "_sb")
            nc.sync.dma_start(t_[:], dr.ap()[:, :])
            ru_sb.append(t_)

        def load_w(dr, in_dim, out_w, name):
            ts = []
            for cch in range(in_dim // P):
                t_ = mk([P, out_w], BF16, f"{name}_c{cch}")
                nc.sync.dma_start(t_[:], dr.ap()[cch * P:(cch + 1) * P, :])
                ts.append(t_)
            return ts

        def load_b(dr, w, name):
            t_ = mk([P, w], F32, name)
            nc.sync.dma_start(t_[:], dr.ap()[0:1, :].to_broadcast([P, w]))
            return t_

        w1_s = load_w(w1, IN_DIM, HID, "w1s")
        b1_s = load_b(b1, HID, "b1s")
        w2_s = load_w(w2, HID, OUT_DIM, "w2s")
        b2_s = load_b(b2, OUT_DIM, "b2s")
        ws, bs = {}, {}
        for k, dr in wd.items():
            ws[k] = load_w(dr, HID, 512 if k.startswith("wkv") else HID, k + "s")
        for k, dr in bd.items():
            bs[k] = load_b(dr, 512 if k.startswith("bkv") else HID, k + "s")

        sb = _stk.enter_context(tc.tile_pool(name="sb", bufs=2))
        pp = _stk.enter_context(tc.tile_pool(name="pp", bufs=2, space="PSUM"))

        zt = mk([P, 16, 264], BF16, "zt")
        nc.vector.memset(zt[:], 0.0)

        def memset_dram(dr, nrows, w, tag):
            nfull = (nrows // P) * P
            v = dr.ap()[0:nfull, :].rearrange("(p a) f -> p a f", p=P)
            a_tot = nfull // P
            a0 = 0
            while a0 < a_tot:
                aa = min(16, a_tot - a0)
                nc.sync.dma_start(v[:, a0:a0 + aa, :], zt[:, 0:aa, 0:w])
                a0 += aa
            if nrows > nfull:
                r = nrows - nfull
                nc.sync.dma_start(dr.ap()[nfull:nrows, :], zt[0:r, 0, 0:w])

        # zero scratch once; scatters overwrite the same rows each layer
        memset_dram(outs_r2, 2 * NR + P, 264, "z1")
        for l in range(L):
            memset_dram(part_u[l], C * UBLK, 264, f"z2{l}")

        def transposed(xt_ap, sz, nch, tag):
            outs = []
            for cch in range(nch):
                tp = pp.tile([P, P], BF16, tag="tp")
                nc.tensor.transpose(
                    out=tp[:, 0:sz],
                    in_=xt_ap[0:sz, cch * P:(cch + 1) * P],
                    identity=ident[0:sz, 0:sz])
                ts = sb.tile([P, P], BF16, tag=f"dts{cch}")
                nc.vector.tensor_copy(ts[:, 0:sz], tp[:, 0:sz])
                outs.append(ts)
            return outs

        def dense(x_dr, nrows, in_dim, jobs, tag):
            """x stored row-major [nrows, in_dim] bf16; PE-transpose tiles."""
            nch = in_dim // P
            for r0, sz in _rows_of(nrows):
                xt = sb.tile([P, in_dim], BF16, tag="dx")
                nc.sync.dma_start(xt[0:sz], x_dr.ap()[r0:r0 + sz, :])
                xT = transposed(xt, sz, nch, tag)
                for wt, finish in jobs:
                    ow = wt[0].shape[-1]
                    ps = pp.tile([P, ow], F32, tag="ps")
                    for cch in range(nch):
                        nc.tensor.matmul(out=ps[0:sz], lhsT=xT[cch][:, 0:sz],
                                         rhs=wt[cch][:],
                                         start=(cch == 0), stop=(cch == nch - 1))
                    finish(ps, r0, sz)

        def dense_T(xT_dr, nrows, in_dim, jobs, tag):
            """x stored TRANSPOSED [in_dim, nrows] bf16; lhsT loads direct."""
            nch = in_dim // P
            for r0, sz in _rows_of(nrows):
                xT = []
                for cch in range(nch):
                    t_ = sb.tile([P, P], BF16, tag=f"dxt{cch}")
                    nc.sync.dma_start(
                        t_[:, 0:sz],
                        xT_dr.ap()[cch * P:(cch + 1) * P, r0:r0 + sz])
                    xT.append(t_)
                for wt, finish in jobs:
                    ow = wt[0].shape[-1]
                    ps = pp.tile([P, ow], F32, tag="ps")
                    for cch in range(nch):
                        nc.tensor.matmul(out=ps[0:sz], lhsT=xT[cch][:, 0:sz],
                                         rhs=wt[cch][:],
                                         start=(cch == 0), stop=(cch == nch - 1))
                    finish(ps, r0, sz)

        def fin_store(bias_t, act, out_dr, off, ow, tag, alpha=0.0,
                      out_f32=False):
            def f(ps, r0, sz):
                ot = sb.tile([P, ow], F32 if out_f32 else BF16, tag="do")
                nc.vector.tensor_add(ot[0:sz], ps[0:sz], bias_t[0:sz, :])
                if act is not None:
                    nc.scalar.activation(out=ot[0:sz], in_=ot[0:sz], func=act,
                                         alpha=alpha)
                nc.sync.dma_start(out_dr.ap()[off + r0: off + r0 + sz, :],
                                  ot[0:sz])
            return f

        # ---- phase 0: input MLP (host-transposed x, no PE transposes)
        for t, x_dr, n in ((0, x_u, NU), (1, x_m, NM), (2, x_r, NR)):
            dense_T(x_dr, n, IN_DIM,
                    [(w1_s, fin_store(b1_s, AF.Lrelu, xs[t][0], 0, HID,
                                      f"p0t{t}", alpha=0.01))], f"p0t{t}")

        def edge_phase(T_n, idx_sb, sel_dr, kv_tab, q_tab, out_tab, tag):
            src_sb, qi_sb, dst_sb = idx_sb
            for b0 in range(T_n // G):
                selt = sb.tile([P, G, P], BF16, tag="selt")
                nc.sync.dma_start(
                    selt[:].rearrange("p g q -> p (g q)"),
                    sel_dr.ap()[b0, :, :])
                kvg = sb.tile([P, G, 512], BF16, tag="kvb")
                qg = sb.tile([P, G, HID], BF16, tag="qb")
                for gi_ in range(G):
                    tj = b0 * G + gi_
                    nc.gpsimd.indirect_dma_start(
                        out=kvg[:, gi_, :], out_offset=None, in_=kv_tab.ap(),
                        in_offset=IndirectOffsetOnAxis(
                            ap=src_sb[:, tj:tj + 1], axis=0))
                    nc.gpsimd.indirect_dma_start(
                        out=qg[:, gi_, :], out_offset=None, in_=q_tab.ap(),
                        in_offset=IndirectOffsetOnAxis(
                            ap=qi_sb[:, tj:tj + 1], axis=0))
                kqb = sb.tile([P, G * HID], F32, tag="kqb")
                nc.vector.tensor_mul(
                    kqb[:].rearrange("p (g c) -> p g c", g=G),
                    kvg[:, :, 0:HID], qg[:])
                lgb = sb.tile([P, G * H], F32, tag="lgb")
                nc.vector.tensor_reduce(
                    out=lgb[:],
                    in_=kqb[:].rearrange("p (x d) -> p x d", d=DH),
                    axis=mybir.AxisListType.X, op=ALU.add)
                rhsb = sb.tile([P, G, 264], BF16, tag="rhsb")
                nc.scalar.activation(
                    out=rhsb[:, :, 0:H],
                    in_=lgb[:].rearrange("p (g h) -> p g h", g=G),
                    func=AF.Exp)
                for gi_ in range(G):
                    tj = b0 * G + gi_
                    nc.vector.tensor_tensor(
                        out=rhsb[:, gi_, H:264].rearrange(
                            "p (h d) -> p h d", h=H),
                        in0=kvg[:, gi_, HID:512].rearrange(
                            "p (h d) -> p h d", h=H),
                        in1=rhsb[:, gi_, 0:H].rearrange(
                            "p (h o) -> p h o", h=H).to_broadcast([P, H, DH]),
                        op=ALU.mult)
                    ssum = pp.tile([P, 512], F32, tag="es")
                    nc.tensor.matmul(out=ssum[:, 0:264], lhsT=selt[:, gi_, :],
                                     rhs=rhsb[:, gi_, :], start=True, stop=True)
                    mo = sb.tile([P, 264], BF16, tag="mo")
                    nc.scalar.activation(out=mo[:], in_=ssum[:, 0:264],
                                         func=AF.Copy)
                    nc.gpsimd.indirect_dma_start(
                        out=out_tab.ap(), in_=mo[:],
                        out_offset=IndirectOffsetOnAxis(
                            ap=dst_sb[:, tj:tj + 1], axis=0),
                        in_offset=None)

        for l in range(L):
            # ---- phase A part 1: movie + user kqv (small), then AGs
            dense(xs[0][l], NU, HID, [
                (ws[f"wkv_t0_l{l}"], fin_store(bs[f"bkv_t0_l{l}"], None,
                                               kv_own[l], NM, 512, f"au{l}")),
                (ws[f"wq_t0_l{l}"], fin_store(bs[f"bq_t0_l{l}"], None,
                                              qu_own[l], 0, HID, f"aq{l}")),
            ], f"au{l}")
            dense(xs[1][l], NM, HID, [
                (ws[f"wkv_t1_l{l}"], fin_store(bs[f"bkv_t1_l{l}"], None,
                                               kv_own[l], 0, 512, f"am{l}")),
            ], f"am{l}")
            nc.gpsimd.collective_compute(
                "AllGather", ALU.bypass, replica_groups=RG,
                ins=[qu_own[l].ap()], outs=[q_uf[l].ap()])
            nc.gpsimd.collective_compute(
                "AllGather", ALU.bypass, replica_groups=RG,
                ins=[kv_own[l].ap()], outs=[kv_src[l].ap()])
            # ---- phase A part 2: review kqv (overlaps the AllGathers)
            dense(xs[2][l], NR, HID, [
                (ws[f"wkv_t2_l{l}"], fin_store(bs[f"bkv_t2_l{l}"], None,
                                               kv_ru[l], 0, 512, f"ar{l}")),
                (ws[f"wq_t2_l{l}"], fin_store(bs[f"bq_t2_l{l}"], None,
                                              q_r[l], 0, HID, f"arq{l}")),
            ], f"ar{l}")

            # ---- phase D first: ru edge tiles need only the (small) q
            # AllGather + local review kv, so they start before the kv
            # AllGather lands; phase C follows; the ReduceScatter is issued
            # after C so its completion wait never stalls C's gathers, and
            # its wire time hides under the review phase-E compute.
            edge_phase(T_D, ru_sb, selD_d, kv_ru[l], q_uf[l], part_u[l],
                       f"d{l}")
            edge_phase(T_C, cs_sb, selC_d, kv_src[l], q_r[l], outs_r2, f"c{l}")
            nc.gpsimd.collective_compute(
                "ReduceScatter", ALU.add, replica_groups=RG,
                ins=[part_u[l].ap()], outs=[red_u[l].ap()])

            # ---- phase E (review first: overlaps the ReduceScatter)
            def fin_blend(bias_t, xs_in, xs_out, t, tag):
                og = omg[(l, t)]
                def f(ps, r0, sz):
                    ot = sb.tile([P, HID], F32, tag="do")
                    nc.vector.tensor_add(ot[0:sz], ps[0:sz], bias_t[0:sz, :])
                    xt2 = sb.tile([P, HID], BF16, tag="dx2")
                    nc.sync.dma_start(xt2[0:sz], xs_in.ap()[r0:r0 + sz, :])
                    ob = sb.tile([P, HID], F32, tag="ob")
                    nc.vector.tensor_scalar_mul(
                        out=ob[0:sz], in0=xt2[0:sz], scalar1=og)
                    nc.vector.tensor_add(ot[0:sz], ot[0:sz], ob[0:sz])
                    oc = sb.tile([P, HID], BF16, tag="oc")
                    nc.vector.tensor_copy(oc[0:sz], ot[0:sz])
                    nc.sync.dma_start(xs_out.ap()[r0:r0 + sz, :], oc[0:sz])
                return f

            def rev_att(r0, sz, tag):
                rt = sb.tile([P, 528], BF16, tag="er")
                nc.sync.dma_start(
                    rt[0:sz],
                    outs_r2.ap()[2 * r0:2 * (r0 + sz), :]
                        .rearrange("(n two) f -> n (two f)", two=2))
                dn = sb.tile([P, 2 * H], F32, tag="edn")
                nc.vector.tensor_scalar_add(
                    out=dn[0:sz].rearrange("p (two f) -> p two f", two=2),
                    in0=rt[0:sz].rearrange("p (two f) -> p two f", two=2)
                        [:, :, 0:H],
                    scalar1=1e-16)
                rd = sb.tile([P, 2 * H], F32, tag="erd")
                nc.vector.reciprocal(out=rd[0:sz], in_=dn[0:sz])
                at = sb.tile([P, HID], F32, tag="ea")
                nc.vector.tensor_tensor(
                    out=at[0:sz].rearrange("p (h d) -> p h d", h=H),
                    in0=rt[0:sz, H:264].rearrange("p (h d) -> p h d", h=H),
                    in1=rd[0:sz, 0:H].rearrange("p (h o) -> p h o", h=H)
                        .to_broadcast([sz, H, DH]),
                    op=ALU.mult)
                a2 = sb.tile([P, HID], F32, tag="ea2")
                nc.vector.tensor_tensor(
                    out=a2[0:sz].rearrange("p (h d) -> p h d", h=H),
                    in0=rt[0:sz, 264 + H:528].rearrange("p (h d) -> p h d", h=H),
                    in1=rd[0:sz, H:2 * H].rearrange("p (h o) -> p h o", h=H)
                        .to_broadcast([sz, H, DH]),
                    op=ALU.mult)
                nc.vector.tensor_add(at[0:sz], at[0:sz], a2[0:sz])
                ag = sb.tile([P, HID], BF16, tag="eag")
                nc.scalar.activation(out=ag[0:sz], in_=at[0:sz], func=AF.Gelu)
                return ag

            def user_att(r0, sz, tag):
                rt = sb.tile([P, 264], BF16, tag="eru")
                nc.sync.dma_start(rt[0:sz], red_u[l].ap()[r0:r0 + sz, :])
                dn = sb.tile([P, H], F32, tag="ednu")
                nc.vector.tensor_scalar_add(
                    out=dn[0:sz], in0=rt[0:sz, 0:H], scalar1=1e-16)
                rd = sb.tile([P, H], F32, tag="erdu")
                nc.vector.reciprocal(out=rd[0:sz], in_=dn[0:sz])
                at = sb.tile([P, HID], F32, tag="eau")
                nc.vector.tensor_tensor(
                    out=at[0:sz].rearrange("p (h d) -> p h d", h=H),
                    in0=rt[0:sz, H:264].rearrange("p (h d) -> p h d", h=H),
                    in1=rd[0:sz].rearrange("p (h o) -> p h o", h=H)
                        .to_broadcast([sz, H, DH]),
                    op=ALU.mult)
                ag = sb.tile([P, HID], BF16, tag="eagu")
                nc.scalar.activation(out=ag[0:sz], in_=at[0:sz], func=AF.Gelu)
                return ag

            for t, n, attf in ((2, NR, rev_att), (0, NU, user_att)):
                wt = ws[f"wa_t{t}_l{l}"]
                fin = fin_blend(bs[f"ba_t{t}_l{l}"], xs[t][l], xs[t][l + 1],
                                t, f"e{t}{l}")
                for r0, sz in _rows_of(n):
                    at = attf(r0, sz, f"e{t}{l}")
                    xT = transposed(at, sz, HID // P, f"e{t}{l}")
                    ps = pp.tile([P, HID], F32, tag="ps2")
                    for cch in range(HID // P):
                        nc.tensor.matmul(out=ps[0:sz], lhsT=xT[cch][:, 0:sz],
                                         rhs=wt[cch][:],
                                         start=(cch == 0), stop=(cch == 1))
                    fin(ps, r0, sz)
            og = omg[(l, 1)]
            bam = bs[f"ba_t1_l{l}"]
            for r0, sz in _rows_of(NM):
                xt = sb.tile([P, HID], BF16, tag="em")
                nc.sync.dma_start(xt[0:sz], xs[1][l].ap()[r0:r0 + sz, :])
                ot = sb.tile([P, HID], F32, tag="em2")
                nc.vector.tensor_scalar_mul(
                    out=ot[0:sz], in0=xt[0:sz], scalar1=og)
                nc.vector.tensor_add(ot[0:sz], ot[0:sz], bam[0:sz, :])
                oc = sb.tile([P, HID], BF16, tag="em3")
                nc.vector.tensor_copy(oc[0:sz], ot[0:sz])
                nc.sync.dma_start(xs[1][l + 1].ap()[r0:r0 + sz, :], oc[0:sz])

        # ---- phase F: output MLP
        for t, y_dr, n in ((0, y_u, NU), (1, y_m, NM), (2, y_r, NR)):
            dense(xs[t][L], n, HID,
                  [(w2_s, fin_store(b2_s, AF.Lrelu, y_dr, 0, OUT_DIM,
                                    f"pft{t}", alpha=0.01, out_f32=True))],
                  f"pft{t}")
        _stk.close()

    nc.finalize()
    return nc


# ---------------------------------------------------------------- entry

_CACHE = {}


def kernel(**inputs):
    import os
    inp = {k: np.asarray(v) for k, v in inputs.items()}
    w = _fold_weights(inp)
    T_C, cs, T_D, ru = _prep_edges(inp)
    omg = {(l, t): w[f"omg_l{l}_t{t}"] for l in range(L) for t in range(3)}

    key = (T_C, T_D)
    if key not in _CACHE:
        _CACHE[key] = build_program(T_C, T_D, omg)
    nc = _CACHE[key]

    cs_names = ["cs_src", "cs_qi", "cs_dst"]
    ru_names = ["ru_src", "ru_qi", "ru_dst"]
    in_maps = []
    for c in range(C):
        m = {
            "x_u": np.ascontiguousarray(
                inp["x_user"][c * NU:(c + 1) * NU].T).astype(BF),
            "x_m": np.ascontiguousarray(
                inp["x_movie"][c * NM:(c + 1) * NM].T).astype(BF),
            "x_r": np.ascontiguousarray(
                inp["x_review"][c * NR:(c + 1) * NR].T).astype(BF),
            "w1": w["w1"], "b1": w["b1"], "w2": w["w2"], "b2": w["b2"],
        }
        arrs, gids = cs[c]
        for n, a in zip(cs_names, arrs):
            m[n] = a
        m["selC"] = _sel_stream(gids)
        arrs, gids = ru[c]
        for n, a in zip(ru_names, arrs):
            m[n] = a
        m["selD"] = _sel_stream(gids)
        for l in range(L):
            for s in range(3):
                m[f"wkv_t{s}_l{l}"] = w[f"wkv_t{s}_l{l}"]
                m[f"bkv_t{s}_l{l}"] = w[f"bkv_t{s}_l{l}"]
            for t in (0, 2):
                for nme in (f"wq_t{t}_l{l}", f"bq_t{t}_l{l}",
                            f"wa_t{t}_l{l}", f"ba_t{t}_l{l}"):
                    m[nme] = w[nme]
            m[f"ba_t1_l{l}"] = w[f"ba_t1_l{l}"]
        in_maps.append(m)

    trace = os.environ.get("BASS_KERNEL_TRACE") == "1"
    res = run_bass_kernel_spmd(nc, in_maps, core_ids=list(range(C)),
                               trace=trace)
    global LAST_RESULTS, LAST_NC
    LAST_RESULTS = res
    LAST_NC = nc
    r = res.results
    yu = np.concatenate([r[c]["y_u"] for c in range(C)], 0)
    ym = np.concatenate([r[c]["y_m"] for c in range(C)], 0)
    yr = np.concatenate([r[c]["y_r"] for c in range(C)], 0)
    return np.concatenate([yu, ym, yr], 0).astype(np.float32)
